# revision 1
# baseline (speedup 1.0000x reference)
"""Trainium2 Bass kernel for nn_LocalFmoeCatEmbedFeedForward.

Strategy (expert-parallel, 8 cores):
  - Host: router (concat -> logits -> softmax -> top-1 gate) + dispatch.
    Tokens are gathered per expert; each expert's tokens are split across
    2 cores (4 experts x 2 = 8 cores). This is the "all-to-all dispatch"
    done host-side since kernel() receives full inputs anyway.
  - Device (per core): H^T = relu(W1 @ X^T + b1) via PE (K=512), then
    Y = H @ W2^T scaled by the gate via ACT per-partition scale.
    Everything stays transposed so no on-device transposes are needed.
  - Host: scatter rows back and add w2_bias contribution if nonzero.

Matmuls run as float32r (single-pass fp32, 1 cycle/row at N>=512) with
fp32 PSUM accumulation.
"""

import os
import sys

sys.path.insert(0, "/opt/trn_rl_repo")

import numpy as np

import concourse.bacc as bacc
import concourse.tile as tile
from concourse import mybir
from concourse import bass_utils

IDIM, EMBED_DIM, NUM_EXPERTS, HIDDEN = 512, 256, 4, 1024
N_CORES = 8
P = 128

_MM_DT = mybir.dt.float32r


def _build_nc(C: int):
    """Build the per-core SPMD program for a token-capacity of C."""
    nc = bacc.Bacc("TRN2", target_bir_lowering=False, debug=False,
                   num_devices=N_CORES)
    f32 = mybir.dt.float32

    xT = nc.dram_tensor("xT", [IDIM, C], _MM_DT, kind="ExternalInput").ap()
    w1p = nc.dram_tensor("w1p", [P, HIDDEN // P * (IDIM // P) * P], _MM_DT,
                         kind="ExternalInput").ap()
    w2p = nc.dram_tensor("w2p", [P, (HIDDEN // P) * IDIM], _MM_DT,
                         kind="ExternalInput").ap()
    b1 = nc.dram_tensor("b1", [P, HIDDEN // P], f32, kind="ExternalInput").ap()
    gate = nc.dram_tensor("gate", [P, C // P], f32, kind="ExternalInput").ap()
    y = nc.dram_tensor("y", [C, IDIM], f32, kind="ExternalOutput").ap()

    K1 = IDIM // P        # 4  k-chunks for GEMM1
    M1 = HIDDEN // P      # 8  m-chunks (H features)
    K2 = HIDDEN // P      # 8  k-chunks for GEMM2
    NT = C // P           # token chunks of 128

    # n-chunks over tokens for GEMM1 (512 wide, last may be partial)
    n_chunks = []
    n0 = 0
    while n0 < C:
        w = min(512, C - n0)
        n_chunks.append((n0, w))
        n0 += w

    with tile.TileContext(nc) as tc:
        with (
            tc.tile_pool(name="xt", bufs=1) as xt_pool,
            tc.tile_pool(name="w", bufs=1) as w_pool,
            tc.tile_pool(name="ht", bufs=1) as ht_pool,
            tc.tile_pool(name="sm", bufs=1) as sm_pool,
            tc.tile_pool(name="yo", bufs=4) as yo_pool,
            tc.tile_pool(name="ps1", bufs=4, space="PSUM") as ps1_pool,
            tc.tile_pool(name="ps2", bufs=4, space="PSUM") as ps2_pool,
        ):
            xT_k = xT.rearrange("(k p) c -> k p c", p=P)

            # m0 weight blocks + small tensors first, then xT streamed per
            # n-chunk: the PE can start after ~1.3MB instead of ~8.5MB.
            b1_sb = sm_pool.tile([P, M1], f32, tag="b1")
            nc.sync.dma_start(b1_sb[:], b1[:])
            gate_sb = sm_pool.tile([P, NT], f32, tag="gate")
            nc.sync.dma_start(gate_sb[:], gate[:])

            # One [128, 32*128] tile holds all w1 (m,k) blocks; the m0
            # group loads as its own DMA so the PE starts early, the rest
            # as one big DMA that can't starve the xt chunk feed.
            w1a = w_pool.tile([P, M1 * K1 * P], _MM_DT, tag="w1a", name="w1a")
            nc.sync.dma_start(w1a[:], w1p[:])

            xt_sb = []
            for k in range(K1):
                t = xt_pool.tile([P, C], _MM_DT, tag=f"xt{k}", name=f"xt{k}")
                xt_sb.append(t)
            for k in range(K1):
                nc.sync.dma_start(xt_sb[k][:, 0:n_chunks[0][1]],
                                  xT_k[k][:, 0:n_chunks[0][1]])
            for (n0, w) in n_chunks[1:]:
                for k in range(K1):
                    nc.sync.dma_start(xt_sb[k][:, n0:n0 + w],
                                      xT_k[k][:, n0:n0 + w])

            w2a = w_pool.tile([P, K2 * IDIM], _MM_DT, tag="w2a", name="w2a")
            nc.sync.dma_start(w2a[:], w2p[:])
            w2_sb = [w2a[:, k * IDIM:(k + 1) * IDIM] for k in range(K2)]

            ht_sb = []
            for m in range(M1):
                ht_sb.append(ht_pool.tile([P, C], _MM_DT, tag=f"ht{m}", name=f"ht{m}"))

            # GEMM1: H^T[m, n] = relu(sum_k W1T[k,m].T @ X^T[k, n] + b1[m])
            # n outer so the first chunk's matmuls only need that chunk's DMA.
            for (n0, w) in n_chunks:
                for m in range(M1):
                    ps = ps1_pool.tile([P, 512], f32, tag="ps1")
                    for k in range(K1):
                        nc.tensor.matmul(
                            ps[:, :w],
                            w1a[:, (m * K1 + k) * P:(m * K1 + k + 1) * P],
                            xt_sb[k][:, n0:n0 + w],
                            start=(k == 0),
                            stop=(k == K1 - 1),
                        )
                    nc.scalar.activation(
                        ht_sb[m][:, n0:n0 + w], ps[:, :w],
                        mybir.ActivationFunctionType.Relu,
                        bias=b1_sb[:, m:m + 1],
                    )

            # GEMM2: Y[t, :] = gate[t] * (sum_k H^T[k,t].T @ W2T[k, :])
            for t in range(NT):
                ps = ps2_pool.tile([P, IDIM], f32, tag="ps2")
                for k in range(K2):
                    nc.tensor.matmul(
                        ps[:],
                        ht_sb[k][:, t * P:(t + 1) * P],
                        w2_sb[k],
                        start=(k == 0),
                        stop=(k == K2 - 1),
                    )
                yt = yo_pool.tile([P, IDIM], f32, tag="yo")
                nc.scalar.activation(
                    yt[:], ps[:],
                    mybir.ActivationFunctionType.Identity,
                    scale=gate_sb[:, t:t + 1],
                )
                nc.sync.dma_start(y[t * P:(t + 1) * P, :], yt[:])

    nc.compile()
    return nc


def kernel(inputs, embed, router_weights, w1_weight, w1_bias, w2_weight,
           w2_bias, mask):
    inputs = np.asarray(inputs, np.float32)
    embed = np.asarray(embed, np.float32)
    router_weights = np.asarray(router_weights, np.float32)
    w1_weight = np.asarray(w1_weight, np.float32)
    w1_bias = np.asarray(w1_bias, np.float32)
    w2_weight = np.asarray(w2_weight, np.float32)
    w2_bias = np.asarray(w2_bias, np.float32)
    mask_f = np.asarray(mask).astype(np.float32)

    K1_H, M1_H = IDIM // P, HIDDEN // P
    B, T, D = inputs.shape
    N = B * T
    x = inputs.reshape(N, D)

    # ---- host router: softmax top-1 over concat(embed, inputs) ----
    router_in = np.concatenate([embed.reshape(N, EMBED_DIM), x], axis=1)
    logits = router_in @ router_weights
    logits -= logits.max(axis=1, keepdims=True)
    p = np.exp(logits)
    p /= p.sum(axis=1, keepdims=True)
    gate_idx = np.argmax(p, axis=1)
    gate_val = p[np.arange(N), gate_idx] * mask_f.reshape(N)

    # ---- dispatch: expert e -> cores 2e, 2e+1 ----
    shard_idx = []
    for e in range(NUM_EXPERTS):
        te = np.nonzero(gate_idx == e)[0]
        h = (len(te) + 1) // 2
        shard_idx.append(te[:h])
        shard_idx.append(te[h:])
    C = max(P, -(-max(len(s) for s in shard_idx) // P) * P)

    nc = _build_nc(C)

    in_maps = []
    for c in range(N_CORES):
        e = c // 2
        idx = shard_idx[c]
        xs = np.zeros((C, D), np.float32)
        xs[: len(idx)] = x[idx]
        gs = np.zeros(C, np.float32)
        gs[: len(idx)] = gate_val[idx]
        in_maps.append({
            "xT": np.ascontiguousarray(xs.T),
            "w1p": np.ascontiguousarray(
                w1_weight[e].T.reshape(K1_H, P, M1_H, P)
                .transpose(1, 2, 0, 3).reshape(P, M1_H * K1_H * P)),
            "w2p": np.ascontiguousarray(
                w2_weight[e].T.reshape(HIDDEN // P, P, IDIM)
                .transpose(1, 0, 2).reshape(P, (HIDDEN // P) * IDIM)),
            "b1": np.ascontiguousarray(w1_bias[e].reshape(HIDDEN // P, P).T),
            "gate": np.ascontiguousarray(gs.reshape(C // P, P).T),
        })

    trace = bool(os.environ.get("KERNEL_TRACE"))
    kw = {}
    if trace:
        bass_utils.upload_artifacts = lambda tmpdir: f"local:{tmpdir}"
        kw = dict(trace=True, trace_cores=list(range(N_CORES)),
                  tmpdir=os.environ.get("KERNEL_TRACE_DIR") or None)
    try:
        res = bass_utils.run_bass_kernel_spmd(
            nc, in_maps, core_ids=list(range(N_CORES)), **kw)
    except Exception:
        res = bass_utils.run_bass_kernel_spmd(
            nc, in_maps, core_ids=list(range(N_CORES)), **kw)
    if trace:
        kernel.exec_time_ns = res.exec_time_ns
        kernel.mean_exec_time_ns = res.mean_exec_time_ns

    out = np.zeros((N, D), np.float32)
    for c in range(N_CORES):
        idx = shard_idx[c]
        out[idx] = res.results[c]["y"][: len(idx)]
    if np.any(w2_bias):
        out += (w2_bias[gate_idx] * gate_val[:, None])
    return out.reshape(B, T, D)



# revision 3
# speedup vs baseline: 1.0813x; 1.0813x over previous
"""Trainium2 Bass kernel for nn_LocalFmoeCatEmbedFeedForward.

Strategy (expert-parallel, 8 cores):
  - Host: router (concat -> logits -> softmax -> top-1 gate) + dispatch.
    Tokens are gathered per expert; each expert's tokens split across 2
    cores (4 experts x 2 = 8 cores). Gate is applied host-side to the
    OUTPUT (y_final = gate * (y_dev + b2)), so the device program needs
    no per-token scaling at all.
  - Device (per core), all matmuls in bf16 (same 1 cyc/row PE rate as
    fp32r but half the DMA bytes — the baseline was DMA-limited at the
    head/tail):
      GEMM1: hT[m, t] = relu(sum_k W1T[k,m].T @ xT[k, t] + b1[m])
      GEMM2: yT[d, t] = sum_k W2T[k,d].T @ hT[k, t]
    Both keep tokens on the free dim, so shard sizes need no 128
    rounding, and GEMM1's output layout directly feeds GEMM2's moving
    operand (no transposes anywhere).
  - Host: scatter rows back, add b2 if nonzero, scale by gate.
"""

import os
import sys

sys.path.insert(0, "/opt/trn_rl_repo")

import numpy as np
import ml_dtypes

import concourse.bacc as bacc
import concourse.tile as tile
from concourse import mybir
from concourse import bass_utils

IDIM, EMBED_DIM, NUM_EXPERTS, HIDDEN = 512, 256, 4, 1024
N_CORES = 8
P = 128
K1 = IDIM // P     # 4   k-blocks for GEMM1
M1 = HIDDEN // P   # 8   m-blocks (h features) = GEMM2's k-blocks
K2 = HIDDEN // P   # 8
D1 = IDIM // P     # 4   d-blocks (output features)

BF16 = mybir.dt.bfloat16
NPBF16 = ml_dtypes.bfloat16


def _build_nc(C: int):
    """Per-core SPMD program for a token capacity of C (any multiple of 32)."""
    nc = bacc.Bacc("TRN2", target_bir_lowering=False, debug=False,
                   num_devices=N_CORES)
    f32 = mybir.dt.float32

    xT = nc.dram_tensor("xT", [P, K1 * C], BF16, kind="ExternalInput").ap()
    w1p = nc.dram_tensor("w1p", [P, M1 * K1 * P], BF16, kind="ExternalInput").ap()
    w2p = nc.dram_tensor("w2p", [P, K2 * D1 * P], BF16, kind="ExternalInput").ap()
    b1 = nc.dram_tensor("b1", [P, M1], f32, kind="ExternalInput").ap()
    y = nc.dram_tensor("y", [P, D1 * C], BF16, kind="ExternalOutput").ap()

    chunks = []
    n0 = 0
    while n0 < C:
        w = min(512, C - n0)
        chunks.append((n0, w))
        n0 += w
    NCH = len(chunks)

    with tile.TileContext(nc) as tc:
        with (
            tc.tile_pool(name="xt", bufs=1) as xt_pool,
            tc.tile_pool(name="w", bufs=1) as w_pool,
            tc.tile_pool(name="ht", bufs=1) as ht_pool,
            tc.tile_pool(name="sm", bufs=1) as sm_pool,
            tc.tile_pool(name="yo", bufs=4) as yo_pool,
            tc.tile_pool(name="ps1", bufs=4, space="PSUM") as ps1_pool,
            tc.tile_pool(name="ps2", bufs=4, space="PSUM") as ps2_pool,
        ):
            b1_sb = sm_pool.tile([P, M1], f32, tag="b1")
            nc.sync.dma_start(b1_sb[:], b1[:])

            w1a = w_pool.tile([P, M1 * K1 * P], BF16, tag="w1a", name="w1a")
            # m0 block alone first so the PE can start early.
            nc.sync.dma_start(w1a[:, 0:K1 * P], w1p[:, 0:K1 * P])

            xt = xt_pool.tile([P, K1 * C], BF16, tag="xt", name="xt")
            xt3 = xt[:].rearrange("p (k c) -> p k c", k=K1)
            xT3 = xT.rearrange("p (k c) -> p k c", k=K1)

            def load_chunk(ci):
                n0, w = chunks[ci]
                nc.sync.dma_start(xt3[:, :, n0:n0 + w], xT3[:, :, n0:n0 + w])

            load_chunk(0)
            nc.sync.dma_start(w1a[:, K1 * P:], w1p[:, K1 * P:])
            if NCH > 1:
                load_chunk(1)
            w2a = w_pool.tile([P, K2 * D1 * P], BF16, tag="w2a", name="w2a")
            nc.sync.dma_start(w2a[:], w2p[:])
            for ci in range(2, NCH):
                load_chunk(ci)

            ht = ht_pool.tile([P, K2 * C], BF16, tag="ht", name="ht")

            def g1(ci):
                n0, w = chunks[ci]
                for m in range(M1):
                    ps = ps1_pool.tile([P, 512], f32, tag="ps1")
                    for k in range(K1):
                        nc.tensor.matmul(
                            ps[:, :w],
                            w1a[:, (m * K1 + k) * P:(m * K1 + k + 1) * P],
                            xt[:, k * C + n0:k * C + n0 + w],
                            start=(k == 0),
                            stop=(k == K1 - 1),
                        )
                    nc.scalar.activation(
                        ht[:, m * C + n0:m * C + n0 + w], ps[:, :w],
                        mybir.ActivationFunctionType.Relu,
                        bias=b1_sb[:, m:m + 1],
                    )

            def g2(ci):
                n0, w = chunks[ci]
                for d in range(D1):
                    ps = ps2_pool.tile([P, 512], f32, tag="ps2")
                    for k in range(K2):
                        nc.tensor.matmul(
                            ps[:, :w],
                            w2a[:, (k * D1 + d) * P:(k * D1 + d + 1) * P],
                            ht[:, k * C + n0:k * C + n0 + w],
                            start=(k == 0),
                            stop=(k == K2 - 1),
                        )
                    yt = yo_pool.tile([P, 512], BF16, tag="yo")
                    nc.scalar.activation(
                        yt[:, :w], ps[:, :w],
                        mybir.ActivationFunctionType.Identity,
                    )
                    nc.gpsimd.dma_start(y[:, d * C + n0:d * C + n0 + w],
                                        yt[:, :w])

            # Software pipeline: GEMM2 of chunk i runs one chunk behind
            # GEMM1 so the ReLU activations have time to drain.
            g1(0)
            for ci in range(1, NCH):
                g1(ci)
                g2(ci - 1)
            g2(NCH - 1)

    nc.compile()
    return nc


def kernel(inputs, embed, router_weights, w1_weight, w1_bias, w2_weight,
           w2_bias, mask):
    inputs = np.asarray(inputs, np.float32)
    embed = np.asarray(embed, np.float32)
    router_weights = np.asarray(router_weights, np.float32)
    w1_weight = np.asarray(w1_weight, np.float32)
    w1_bias = np.asarray(w1_bias, np.float32)
    w2_weight = np.asarray(w2_weight, np.float32)
    w2_bias = np.asarray(w2_bias, np.float32)
    mask_f = np.asarray(mask).astype(np.float32)

    B, T, D = inputs.shape
    N = B * T
    x = inputs.reshape(N, D)

    # ---- host router: softmax top-1 over concat(embed, inputs) ----
    router_in = np.concatenate([embed.reshape(N, EMBED_DIM), x], axis=1)
    logits = router_in @ router_weights
    logits -= logits.max(axis=1, keepdims=True)
    p = np.exp(logits)
    p /= p.sum(axis=1, keepdims=True)
    gate_idx = np.argmax(p, axis=1)
    gate_val = p[np.arange(N), gate_idx] * mask_f.reshape(N)

    # ---- dispatch: expert e -> cores 2e, 2e+1 ----
    shard_idx = []
    for e in range(NUM_EXPERTS):
        te = np.nonzero(gate_idx == e)[0]
        h = (len(te) + 1) // 2
        shard_idx.append(te[:h])
        shard_idx.append(te[h:])
    C = max(32, -(-max(len(s) for s in shard_idx) // 32) * 32)

    nc = _build_nc(C)

    in_maps = []
    for c in range(N_CORES):
        e = c // 2
        idx = shard_idx[c]
        xs = np.zeros((C, D), np.float32)
        xs[: len(idx)] = x[idx]
        # xT [P, K1*C]: [p, k, t] = x[t, k*128+p]
        xTp = np.ascontiguousarray(
            xs.T.reshape(K1, P, C).transpose(1, 0, 2).reshape(P, K1 * C)
        ).astype(NPBF16)
        # w1p [P, M1*K1*P]: [p, m, k, j] = W1T[k*128+p, m*128+j]
        w1p = np.ascontiguousarray(
            w1_weight[e].T.reshape(K1, P, M1, P)
            .transpose(1, 2, 0, 3).reshape(P, M1 * K1 * P)
        ).astype(NPBF16)
        # w2p [P, K2*D1*P]: [p, k, d, j] = W2T[k*128+p, d*128+j]
        w2p = np.ascontiguousarray(
            w2_weight[e].T.reshape(K2, P, D1, P)
            .transpose(1, 0, 2, 3).reshape(P, K2 * D1 * P)
        ).astype(NPBF16)
        b1p = np.ascontiguousarray(w1_bias[e].reshape(M1, P).T)
        in_maps.append({"xT": xTp, "w1p": w1p, "w2p": w2p, "b1": b1p})

    trace = bool(os.environ.get("KERNEL_TRACE"))
    kw = {}
    if trace:
        bass_utils.upload_artifacts = lambda tmpdir: f"local:{tmpdir}"
        kw = dict(trace=True, trace_cores=list(range(N_CORES)),
                  tmpdir=os.environ.get("KERNEL_TRACE_DIR") or None)
    try:
        res = bass_utils.run_bass_kernel_spmd(
            nc, in_maps, core_ids=list(range(N_CORES)), **kw)
    except Exception:
        res = bass_utils.run_bass_kernel_spmd(
            nc, in_maps, core_ids=list(range(N_CORES)), **kw)
    if trace:
        kernel.exec_time_ns = res.exec_time_ns
        kernel.mean_exec_time_ns = res.mean_exec_time_ns

    out = np.zeros((N, D), np.float32)
    for c in range(N_CORES):
        idx = shard_idx[c]
        # y [P, D1*C]: [p, d, t] = yT[d*128+p, t] = y_row[t, d*128+p]
        arr = np.asarray(res.results[c]["y"]).astype(np.float32)
        rows = arr.reshape(P, D1, C).transpose(2, 1, 0).reshape(C, D1 * P)
        out[idx] = rows[: len(idx)]
    if np.any(w2_bias):
        out += w2_bias[gate_idx]
    out *= gate_val[:, None]
    return out.reshape(B, T, D)
